# revision 25
# baseline (speedup 1.0000x reference)
"""Causal self-attention (B=4, S=2048, D=2048, H=16) on 8 Trainium2 cores.

Sharding: core c -> (batch b = c//2, head-half = c%2, i.e. 8 of 16 heads).
Megatron-style: Wq/Wk/Wv column-parallel (8 heads' rows), Wo row-parallel
(matching 1024 columns).  Each core emits a partial (S, D) output for its
batch; host sums the two half partials per batch and adds bo.

v3 design: all matmul operands bf16 (same PE rate as fp32r, half DMA/
SBUF), K^T/Q^T/V SBUF-resident end to end (no DRAM spill round-trips),
stage 2 qc-outer/head-inner with the out-projection fused per q-chunk.
Per unit (h, qc): all score matmuls first, then all context matmuls
(keeps the in-order PE queue off the exp/mask critical path).  Softmax
denominator: bf16 pairwise-tree adds on DVE (4x mode), per-128-q-column
partition-reduce matmuls, small [128,8] reciprocal, DMA broadcast
round-trip.  fp32 PSUM accumulation everywhere; causal-trimmed diagonal
score/context matmuls; PE warm-up matmuls at t=0 for the HAM clock gate.

Device pipeline per core (S=2048, DK=128, 8 local heads):
  Stage 1 (two 4-head passes): QKV projections.
    Q^T,K^T per head in [dk, s]; V in [s, dv] chunks -> all in SBUF bf16.
  Stage 2 per q-chunk (512), heads inner:
    S^T tile [k,q] = K^T_chunk.T @ Q^T   (bf16, diag tiles suffix-only)
    es = Exp(S^T / sqrt(dk)) -> bf16     (no max-subtraction; scores~N(0,1))
    causal mask + prefix zero-fill: gpsimd affine_select on diag tiles
    ctx^T [dv,q] = sum_k V_chunk.T @ es (PE, fp32 PSUM)
    acc = tree-sum(es) bf16; dent[q,2] = acc_chunk.T @ ones; rcp; bcast
    ctx_sb = ctx^T * rcp -> bf16 (DVE)
    out-proj for the q-chunk: out[q,e] = sum_h ctx_sb_h.T @ WoT_h
    (h-outer over 4 resident e-chunk PSUM banks), DVE copy, DMA out.
"""

import math

import numpy as np

import concourse.bass as bass
import concourse.mybir as mybir
from concourse.bass_utils import run_bass_kernel_spmd
from concourse.tile import TileContext

B, S, D, H = 4, 2048, 2048, 16
DK = 128
NCORES = 8
HPC = H // 2          # 8 heads per core
MLOC = HPC * DK       # 1024 local head dims

F32 = mybir.dt.float32
F32R = mybir.dt.float32r
BF16 = mybir.dt.bfloat16
AF = mybir.ActivationFunctionType


def split_excess_waits(nc, max_waits=1):
    """walrus in this container accepts at most one sem-wait per instruction;
    move excess waits onto wait-only EventSemaphore insts inserted before."""
    ctr = 0
    for f in nc.m.functions:
        for bb in f.blocks:
            new = []
            changed = False
            for inst in bb.instructions:
                si = inst.sync_info
                if si is not None and si.on_wait and len(si.on_wait) > max_waits:
                    changed = True
                    waits = list(si.on_wait)
                    for w in waits[:-max_waits]:
                        ctr += 1
                        ev = mybir.InstEventSemaphore(
                            name=f"waitsplit-{ctr}", ins=[], outs=[],
                            sync_info=mybir.SyncInfo(on_wait=[w], on_update=[]))
                        ev.engine = inst.engine
                        new.append(ev)
                    si.on_wait = waits[-max_waits:]
                new.append(inst)
            if changed:
                bb.instructions = new
    return ctr


def build_nc(seq=S):
    """One core's program: full attention for 1 batch x 8 heads."""
    assert seq % 512 == 0
    NSC = seq // 512          # 512-wide q chunks
    NKC = seq // 128          # 128-wide k chunks
    SCALE = 1.0 / math.sqrt(DK)

    nc = bass.Bass()
    xt = nc.declare_dram_parameter("xt", [D, seq], BF16, isOutput=False)
    wqt = nc.declare_dram_parameter("wqt", [D, MLOC], BF16, isOutput=False)
    wkt = nc.declare_dram_parameter("wkt", [D, MLOC], BF16, isOutput=False)
    wvt = nc.declare_dram_parameter("wvt", [D, MLOC], BF16, isOutput=False)
    wot = nc.declare_dram_parameter("wot", [MLOC, D], BF16, isOutput=False)
    bqt = nc.declare_dram_parameter("bqt", [DK, HPC], F32, isOutput=False)
    bkt = nc.declare_dram_parameter("bkt", [DK, HPC], F32, isOutput=False)
    bvv = nc.declare_dram_parameter("bvv", [MLOC], BF16, isOutput=False)
    ident4 = nc.declare_dram_parameter("ident4", [128, 512], BF16, isOutput=False)
    out = nc.declare_dram_parameter("out", [seq, D], F32, isOutput=True)

    xt_r = xt.rearrange("(dc p) s -> p dc s", p=128)      # [128, 16, seq]
    wqt_r = wqt.rearrange("(dc p) m -> p dc m", p=128)    # [128, 16, 1024]
    wkt_r = wkt.rearrange("(dc p) m -> p dc m", p=128)
    wvt_r = wvt.rearrange("(dc p) m -> p dc m", p=128)
    wot_r = wot.rearrange("(hc p) e -> p hc e", p=128)    # [128, 8, 2048]

    with TileContext(nc) as tc:
        with tc.tile_pool(name="big", bufs=1) as bpool, \
             tc.tile_pool(name="const", bufs=1) as cpool:
            # SBUF-resident per-head tensors (bf16)
            kT = bpool.tile([128, HPC, seq], BF16, name="kT")    # [dk, h, s]
            qT = bpool.tile([128, HPC, seq], BF16, name="qT")    # [dk, h, s]
            vA = bpool.tile([128, NKC, HPC, 128], BF16, name="vA")  # [s, kc, h, dv]

            ones_f = cpool.tile([DK, 2], BF16)
            nc.vector.memset(ones_f[:], 1.0)
            warm = cpool.tile([128, 128], BF16)
            nc.gpsimd.memset(warm[:], 0.0)
            bq_sb = cpool.tile([DK, HPC], F32)
            nc.sync.dma_start(out=bq_sb[:], in_=bqt[:])
            bk_sb = cpool.tile([DK, HPC], F32)
            nc.sync.dma_start(out=bk_sb[:], in_=bkt[:])
            bv_sb = cpool.tile([128, 2, 512], BF16)
            for p_ in range(2):
                nc.sync.dma_start(
                    out=bv_sb[:, p_, :],
                    in_=bvv[p_*512:(p_+1)*512].partition_broadcast(128))

            # ---------------- Stage 1: QKV projections ----------------
            with tc.tile_pool(name="s1w", bufs=1) as wpool, \
                 tc.tile_pool(name="s1x", bufs=2) as xpool, \
                 tc.tile_pool(name="s1qk", bufs=3, space="PSUM") as qkp, \
                 tc.tile_pool(name="s1v", bufs=2, space="PSUM") as vps:
                # PE warm-up: ~3.5us of junk matmuls so the HAM clock gate
                # flips to 8/8 before the first real matmul arrives.
                wps = qkp.tile([128, 512], F32, tag="qk", name="wps")
                for i in range(160):
                    nc.tensor.matmul(wps[:, 0:128], warm[:], warm[:],
                                     start=(i == 0), stop=(i == 159))
                xin0 = None
                for p_ in range(2):           # head-half pass: heads 4p..4p+3
                    # first x chunk ahead of the weight queue -> early start
                    wq_sb = wpool.tile([128, 16, 512], BF16, tag="wq")
                    wk_sb = wpool.tile([128, 16, 512], BF16, tag="wk")
                    wv_sb = wpool.tile([128, 16, 512], BF16, tag="wv")
                    if p_ == 0:
                        xin0 = xpool.tile([128, 16, 512], BF16, tag="xin",
                                          name="xin0")
                        for c8 in range(8):
                            nc.sync.dma_start(
                                out=xin0[:, c8*2:(c8+1)*2, :],
                                in_=xt_r[:, c8*2:(c8+1)*2, 0:512])
                            nc.sync.dma_start(
                                out=wq_sb[:, c8*2:(c8+1)*2, :],
                                in_=wqt_r[:, c8*2:(c8+1)*2, 0:512])
                    else:
                        for c4 in range(4):
                            nc.sync.dma_start(
                                out=wq_sb[:, c4*4:(c4+1)*4, :],
                                in_=wqt_r[:, c4*4:(c4+1)*4, p_*512:(p_+1)*512])
                    for c4 in range(4):
                        nc.sync.dma_start(
                            out=wk_sb[:, c4*4:(c4+1)*4, :],
                            in_=wkt_r[:, c4*4:(c4+1)*4, p_*512:(p_+1)*512])
                    for c4 in range(4):
                        nc.sync.dma_start(
                            out=wv_sb[:, c4*4:(c4+1)*4, :],
                            in_=wvt_r[:, c4*4:(c4+1)*4, p_*512:(p_+1)*512])
                    for sc in range(NSC):
                        if p_ == 0 and sc == 0:
                            xin = xin0
                        else:
                            xin = xpool.tile([128, 16, 512], BF16, tag="xin")
                            for c4 in range(4):
                                nc.sync.dma_start(
                                    out=xin[:, c4*4:(c4+1)*4, :],
                                    in_=xt_r[:, c4*4:(c4+1)*4, sc*512:(sc+1)*512])
                        for hh in range(4):
                            h = p_ * 4 + hh
                            qps = qkp.tile([128, 512], F32, tag="qk")
                            for dc in range(16):
                                nc.tensor.matmul(
                                    qps[:], wq_sb[:, dc, hh*128:(hh+1)*128],
                                    xin[:, dc, :], start=(dc == 0), stop=(dc == 15))
                            nc.scalar.activation(qT[:, h, sc*512:(sc+1)*512],
                                                 qps[:], AF.Identity,
                                                 bias=bq_sb[:, h:h+1], scale=1.0)

                            kps = qkp.tile([128, 512], F32, tag="qk")
                            for dc in range(16):
                                nc.tensor.matmul(
                                    kps[:], wk_sb[:, dc, hh*128:(hh+1)*128],
                                    xin[:, dc, :], start=(dc == 0), stop=(dc == 15))
                            nc.scalar.activation(kT[:, h, sc*512:(sc+1)*512],
                                                 kps[:], AF.Identity,
                                                 bias=bk_sb[:, h:h+1], scale=1.0)
                        # V for this pass: [k, dv] chunks (4 heads' dv)
                        for kc in range(4):
                            vp = vps.tile([128, 512], F32, tag="v")
                            for dc in range(16):
                                nc.tensor.matmul(
                                    vp[:], xin[:, dc, kc*128:(kc+1)*128],
                                    wv_sb[:, dc, :], start=(dc == 0), stop=(dc == 15))
                            nc.vector.tensor_add(
                                vA[:, sc*4+kc, p_*4:(p_+1)*4, :],
                                vp[:].rearrange("p (h v) -> p h v", v=128),
                                bv_sb[:, p_, :].rearrange("p (h v) -> p h v", v=128))

            # ------- Stage 2+3: attention + fused out-projection -------
            # Three-station software pipeline per unit u=(qc,h):
            #   FRONT(u): score matmuls (paired PSUM tiles) + paired exps
            #             + causal masks          [PE+ACT+GPSIMD]
            #   BACK(u):  ctx matmuls, denominator tree, dent matmuls,
            #             reciprocal, diag build  [PE+DVE]
            #   NORM(u):  PE broadcast of 1/den, normalize into ctx_sb
            # emitted as ... NORM(u-2) | outproj(qc-1) | FRONT(u) | BACK(u-1)
            # so no PE instruction ever waits on a just-issued ACT/DVE op.
            with tc.tile_pool(name="s2wo", bufs=1) as wopool, \
                 tc.tile_pool(name="s2es", bufs=2) as espool, \
                 tc.tile_pool(name="s2tmp", bufs=2) as tmppool, \
                 tc.tile_pool(name="s2rcp", bufs=2) as rcpool, \
                 tc.tile_pool(name="s2cx", bufs=2) as cxpool, \
                 tc.tile_pool(name="s3o", bufs=2) as opool, \
                 tc.tile_pool(name="psp", bufs=2, space="PSUM") as psp, \
                 tc.tile_pool(name="pcd", bufs=2, space="PSUM") as pcd, \
                 tc.tile_pool(name="pop", bufs=2, space="PSUM") as pop:
                wo_sb = wopool.tile([128, HPC, D], BF16, name="wo_sb")
                for c4 in range(4):
                    nc.sync.dma_start(
                        out=wo_sb[:, c4*2:(c4+1)*2, :],
                        in_=wot_r[:, c4*2:(c4+1)*2, :])
                ident4_sb = cpool.tile([128, 512], BF16, name="ident4_sb")
                nc.sync.dma_start(out=ident4_sb[:], in_=ident4[:])
                ones128 = cpool.tile([128, 128], BF16, name="ones128")
                nc.vector.memset(ones128[:], 1.0)

                units = [(qc, h) for qc in range(NSC) for h in range(HPC)]
                NU = len(units)
                st = {}      # u -> (es, ctxp, diag4)
                ctxs = {}    # qc -> ctx_sb tile

                def emit_outproj(oqc, ss):
                    octx = ctxs[oqc]
                    for ep in range(2):
                        ops = [pop.tile([128, 512], F32, tag="op",
                                        name=f"op{ei}") for ei in range(2)]
                        for h in range(HPC):
                            for ei in range(2):
                                nc.tensor.matmul(
                                    ops[ei][:], octx[:, h, ss*128:(ss+1)*128],
                                    wo_sb[:, h, (ep*2+ei)*512:(ep*2+ei+1)*512],
                                    start=(h == 0), stop=(h == HPC - 1))
                        for ei in range(2):
                            o_sb = opool.tile([128, 512], F32, tag="o")
                            nc.vector.tensor_copy(o_sb[:], ops[ei][:])
                            nc.sync.dma_start(
                                out=out[oqc*512+ss*128:oqc*512+(ss+1)*128,
                                        (ep*2+ei)*512:(ep*2+ei+1)*512],
                                in_=o_sb[:])

                def front(u):
                    qc, h = units[u]
                    nk = 4 * qc + 4
                    if h == 0:
                        ctxs[qc] = cxpool.tile([128, HPC, 512], BF16,
                                               tag="ctxq", name=f"ctx{qc}")
                    es = espool.tile([128, NKC, 512], BF16, tag="es")
                    es_f = es.rearrange("p a b -> p (a b)")
                    for kp in range(nk // 2):
                        sp2 = psp.tile([128, 2, 512], F32, tag="sp")
                        lo0 = 0
                        for t_ in range(2):
                            kc = 2 * kp + t_
                            j = kc - 4 * qc
                            lo = 128 * j if j > 0 else 0
                            if t_ == 0:
                                lo0 = lo
                            nc.tensor.matmul(
                                sp2[:, t_, lo:], kT[:, h, kc*128:(kc+1)*128],
                                qT[:, h, qc*512+lo:(qc+1)*512],
                                start=True, stop=True)
                        sp_f = sp2.rearrange("p a b -> p (a b)")
                        nc.scalar.activation(
                            es_f[:, 2*kp*512+lo0:(2*kp+2)*512],
                            sp_f[:, lo0:], AF.Exp, bias=0.0, scale=SCALE)
                        for t_ in range(2):
                            kc = 2 * kp + t_
                            j = kc - 4 * qc
                            if j >= 0:   # diag: causal mask + zero the prefix
                                nc.gpsimd.affine_select(
                                    out=es[:, kc, :], in_=es[:, kc, :],
                                    compare_op=mybir.AluOpType.is_ge,
                                    fill=0.0, base=-128 * j,
                                    pattern=[[1, 512]], channel_multiplier=-1)
                    st[u] = [es, None, None]

                def back(u):
                    qc, h = units[u]
                    nk = 4 * qc + 4
                    es = st[u][0]
                    ctxp = pcd.tile([128, 512], F32, tag="cd")
                    for kc in range(nk):
                        j = kc - 4 * qc
                        lo = 128 * j if j > 0 else 0
                        nc.tensor.matmul(
                            ctxp[:, lo:], vA[:, kc, h, :], es[:, kc, lo:],
                            start=(kc == 0), stop=(kc == nk - 1))
                    # denominator: folded halves tree, few wide DVE ops
                    half = nk // 2
                    t8 = tmppool.tile([128, NKC // 2, 512], BF16, tag="tmp")
                    nc.vector.tensor_add(t8[:, 0:half, :], es[:, 0:half, :],
                                         es[:, half:nk, :])
                    n = half
                    while n > 1:
                        m = n // 2
                        nc.vector.tensor_add(t8[:, 0:m, :], t8[:, 0:m, :],
                                             t8[:, m:2*m, :])
                        if n % 2:   # fold the odd leftover tile into slot 0
                            nc.vector.tensor_add(t8[:, 0, :], t8[:, 0, :],
                                                 t8[:, n-1, :])
                        n = m
                    st[u][1] = ctxp
                    st[u][2] = t8

                def den(u):
                    # dent matmuls a full block after the tree -> no PE wait
                    t8 = st[u][2]
                    acc = t8[:, 0, :]
                    dent = psp.tile([128, 2, 512], F32, tag="sp", name="dent")
                    for qs in range(4):
                        nc.tensor.matmul(
                            dent[:, 0, 2*qs:2*qs+2],
                            acc[:, qs*128:(qs+1)*128],
                            ones_f[:], start=True, stop=True)
                    rcpt = rcpool.tile([128, 8], F32, tag="rcpt")
                    nc.vector.reciprocal(rcpt[:], dent[:, 0, 0:8])
                    # diag4[c, qs*128+q'] = rcp[qs,q']*I(c==q') (bf16)
                    diag4 = rcpool.tile([128, 4, 128], BF16, tag="diag4")
                    for qs in range(4):
                        nc.vector.tensor_scalar_mul(
                            diag4[:, qs, :], ident4_sb[:, qs*128:(qs+1)*128],
                            rcpt[:, 2*qs:2*qs+1])
                    st[u][2] = diag4

                def norm(u):
                    qc, h = units[u]
                    es, ctxp, diag4 = st.pop(u)
                    # rcpb[p, q] = rcp[q] : ones128.T @ diag4  (PE broadcast)
                    rsp = psp.tile([128, 2, 512], F32, tag="sp", name="rsp")
                    nc.tensor.matmul(rsp[:, 0, :], ones128[:],
                                     diag4.rearrange("p a b -> p (a b)"),
                                     start=True, stop=True)
                    rcpb = rcpool.tile([128, 512], F32, tag="rcpb")
                    nc.vector.tensor_copy(rcpb[:], rsp[:, 0, :])
                    nc.vector.tensor_mul(ctxs[qc][:, h, :], ctxp[:], rcpb[:])

                for i in range(NU + 3):
                    if i - 3 >= 0:
                        norm(i - 3)
                    if 0 <= i - 2 < NU:
                        den(i - 2)
                    if i < NU:
                        qc, h = units[i]
                        if qc > 0 and h in (2, 4, 6, 7):
                            emit_outproj(qc - 1, (2, 4, 6, 7).index(h))
                        front(i)
                    if 0 <= i - 1 < NU:
                        back(i - 1)
                # tail: out-projection of the last q-chunk
                for ss in range(4):
                    emit_outproj(NSC - 1, ss)
    split_excess_waits(nc)
    return nc


_NC_CACHE = {}


def _get_nc(seq):
    if seq not in _NC_CACHE:
        _NC_CACHE[seq] = build_nc(seq)
    return _NC_CACHE[seq]


def make_in_maps(x, Wq, bq, Wk, bk, Wv, bv, Wo, bo, seq=S, nb=B):
    import ml_dtypes
    bf16 = ml_dtypes.bfloat16
    f32 = np.float32
    in_maps = []
    for c in range(NCORES):
        b = c // 2
        half = c % 2
        sl = slice(half * MLOC, (half + 1) * MLOC)
        in_maps.append({
            "xt": np.ascontiguousarray(x[b].T.astype(bf16)),
            "wqt": np.ascontiguousarray(Wq[sl, :].T.astype(bf16)),
            "wkt": np.ascontiguousarray(Wk[sl, :].T.astype(bf16)),
            "wvt": np.ascontiguousarray(Wv[sl, :].T.astype(bf16)),
            "wot": np.ascontiguousarray(Wo[:, sl].T.astype(bf16)),
            "bqt": np.ascontiguousarray(bq[sl].reshape(HPC, DK).T, dtype=f32),
            "bkt": np.ascontiguousarray(bk[sl].reshape(HPC, DK).T, dtype=f32),
            "bvv": np.ascontiguousarray(bv[sl].astype(bf16)),
            "ident4": np.ascontiguousarray(
                np.tile(np.eye(128, dtype=np.float32), (1, 4)).astype(bf16)),
        })
    return in_maps


def run(inputs, trace=False, trace_kwargs=None):
    x = np.asarray(inputs["x"], dtype=np.float32)
    nb, seq, d = x.shape
    nc = _get_nc(seq)
    in_maps = make_in_maps(
        x, np.asarray(inputs["Wq"]), np.asarray(inputs["bq"]),
        np.asarray(inputs["Wk"]), np.asarray(inputs["bk"]),
        np.asarray(inputs["Wv"]), np.asarray(inputs["bv"]),
        np.asarray(inputs["Wo"]), np.asarray(inputs["bo"]), seq=seq, nb=nb)
    res = run_bass_kernel_spmd(nc, in_maps, list(range(NCORES)), trace=trace,
                               **(trace_kwargs or {}))
    bo = np.asarray(inputs["bo"], dtype=np.float32)
    out = np.empty((nb, seq, d), dtype=np.float32)
    for b in range(nb):
        out[b] = res.results[2*b]["out"] + res.results[2*b+1]["out"] + bo
    return out, res


def kernel(**inputs):
    out, _ = run(inputs, trace=False)
    return out
